# revision 27
# baseline (speedup 1.0000x reference)
"""Trainium2 Bass kernel for CDRExtractor (segment_reduce).

Input : segmentation_mask (64, 3, 512, 512) fp32
Output: (64, 5) fp32 = [cdr, disc_mean, cup_mean, disc_mean, cup_mean]

Sharding: pure data parallel, 8 samples per core across 8 cores.

v3 design, 68.4us (v2 was 76.8us).  What changed and why:
  - SWDGE (gpsimd) casting DMA loads fp32->bf16 at HALF the queue cost
    (the cost model prices a DMA by OUTPUT bytes per partition, and the
    CoreSim model charges each DMA to its issuing engine queue).  The
    24-plane shard splits: ~14 planes fp32 on the SP queue, ~10 planes
    bf16-cast on the Pool queue (samples 0,3 fully cast; 2,4,6 "mixed"
    with x2 cast).  DMA queue time drops 75.8us -> ~62us total.
  - Everything after exp works in f-space (exp is monotonic):
    d1 = rowcount[f1 > max(f2,1)], A = rowcount[max(f1,f2) > 1], and
    d2 = A - d1 exactly (A counts argmax in {1,2}).  A's count is a 4x
    tensor_scalar accumulate off a 2x TT max, which replaces d2's 1x
    fused STT.  f-space also lets exp run IN PLACE over the subtract
    output (cast samples: in place over the X tiles), killing the
    separate T tiles and ~30% of SBUF traffic/pressure.
  - ACT does the exps (one 4096-wide instr per unchunked sample) plus
    ln/exp denominators for ~5 samples; the other denominators run on
    DVE (ts_add 4x + reciprocal 1x + sum_r accumulate 4x).  Consts DMA
    and the act-table warm hide in ACT's idle ramp.
  - Real-ISA limits found on the axon path (the cost model is laxer):
    Pool/gpsimd cannot run TensorScalarPtr at all (no tensor_scalar /
    scalar_tensor_tensor / accum_out) and Pool TT supports only
    add/subtract/mult (no max); TT divide and DMA cast+accum are
    rejected everywhere.  So every accumulation (p1, sum_r, A, d1) and
    every max lives on DVE; Pool gets subs/sadd/p1-mult/g-subs.
  - Tail without PE: gpsimd tensor_reduce(C axis) does the partition
    sums/maxes directly (add/avg/max only - ymin is computed as a max
    of negated iota columns), removing both PE transposes, the
    identity/ones consts, and PSUM.  heights: rowcount>0.5 penalty +
    iota min/max as before; cdr = h_cup/(h_disc+1e-6); means via
    sum p1 and the identity sum p2 = HW - sum r - sum p1.
  - Schedule: software pipeline with lags front/mid/back = 2/3/4,
    per-iteration emission order back,mid,front,loads; s0 chunked at
    [e0,e1,(e2e3)] for the ramp; s1's front at halves; samples 5,6,7
    woven at half granularity near the drain with their loads
    interleaved from iterations 3-6.  Engine busy (CoreSim): Pool 60.6
    / DVE 57.4 / ACT 57.3 / SP 47.9; makespan 68.4us.

Numerics: bf16 internals, fp32 accumulators.  Rel err vs fp32
reference 2.05e-04 (gate 2e-2), HW-verified via test.py on the axon
run path: HW exec 68395 ns.
"""

import numpy as np
from contextlib import ExitStack

B, C, H, W = 64, 3, 512, 512
NCORES = 8
SPC = B // NCORES      # samples per core = 8
NB = H // 128          # 128-row blocks = 4
HW = float(H * W)

_CACHE = {}

# ---- per-sample engine/config tables ----
CFG = dict(
    # load: 'cast' (Pool SWDGE bf16) | 'sp' (fp32 on SP) | 'mixed'
    # (x0,x1 fp32 on SP; x2 cast on Pool)
    load={0: "cast", 1: "sp", 2: "mixed", 3: "cast", 4: "mixed", 5: "sp",
          6: "mixed", 7: "sp"},
    denom={0: "dve", 1: "act", 2: "act", 3: "dve", 4: "act", 5: "split",
           6: "act", 7: "act"},
    subs={0: "dve", 1: "pool", 2: "pool", 3: "dve", 4: "pool",
          5: "pool", 6: "pool", 7: "pool"},
    sadd={0: "dve", 1: "pool", 2: "pool", 3: "dve", 4: "pool",
          5: "pool", 6: "pool", 7: "pool"},
    p1={0: "pool", 1: "dve", 2: "pool", 3: "pool", 4: "dve", 5: "dve",
        6: "pool", 7: "dve"},
    mEng={0: "dve", 1: "dve", 2: "dve", 3: "dve", 4: "dve", 5: "dve",
          6: "dve", 7: "dve"},
    # d1 mode: 'fused' (DVE STT 1x) | 'B' (DVE ts_max + g-sub TT + is_gt)
    d1={0: "fused", 1: "B", 2: "fused", 3: "fused", 4: "B", 5: "B",
        6: "B", 7: "B"},
    d1_g_eng={1: "pool", 4: "pool", 5: "pool", 6: "pool", 7: "pool"},
    lag_front=2, lag_mid=3, lag_back=4,
    chunk_head=True,     # s0 loads/front at half-plane granularity
    chunk_s1=True,       # s1 subs/exps at half granularity (ramp)
    weave=(5, 6, 7),     # samples staged at half granularity near the end
    weave_lag={5: 5, 6: 6, 7: 7},
    il_start={5: 3, 6: 4, 7: 5},   # iteration when woven loads interleave
    order="bmfl",        # emission order within an iteration
    tail_tt="pool",      # engine for tail TT ops
)


def _build():
    import concourse.bass as bass
    import concourse.bacc as bacc
    import concourse.mybir as mybir
    from concourse.tile import TileContext

    if not _CACHE.get("act_patch"):
        _orig_tables = bacc.get_activation_tables

        def _only_ln_exp(arch):
            t = _orig_tables(arch)
            keep = "natural_log_exp_and_others"
            return {k: (v if k == keep else set()) for k, v in t.items()}

        bacc.get_activation_tables = _only_ln_exp
        _CACHE["act_patch"] = True

    f32 = mybir.dt.float32
    bf16 = mybir.dt.bfloat16
    Alu = mybir.AluOpType
    AFT = mybir.ActivationFunctionType
    X_AX = mybir.AxisListType.X
    C_AX = mybir.AxisListType.C

    nc = bacc.Bacc()
    x = nc.dram_tensor("x", (SPC, C, H, W), f32, kind="ExternalInput")
    iota_in = nc.dram_tensor("iota", (128, 64), f32, kind="ExternalInput")
    out = nc.dram_tensor("out", (5, SPC), f32, kind="ExternalOutput")

    s_last = SPC - 1

    def is_cast(s):
        return CFG["load"][s] == "cast"

    with TileContext(nc) as tc, ExitStack() as ctx:
        QENG = dict(sp=nc.sync, act=nc.scalar, pool=nc.gpsimd,
                    dve=nc.vector)
        cpool = ctx.enter_context(tc.tile_pool(name="consts", bufs=1))
        apool = ctx.enter_context(tc.tile_pool(name="accs", bufs=1))
        xcpool = ctx.enter_context(tc.tile_pool(name="xc", bufs=3))
        xfpool = ctx.enter_context(tc.tile_pool(name="xf", bufs=2))
        fpool = ctx.enter_context(tc.tile_pool(name="fmain", bufs=3))
        mpool = ctx.enter_context(tc.tile_pool(name="mid", bufs=3))
        bpool = ctx.enter_context(tc.tile_pool(name="bck", bufs=2))
        rpool = ctx.enter_context(tc.tile_pool(name="rr", bufs=3))

        # act-table warm + consts ride the ACT queue's idle ramp
        warm = cpool.tile([1, 16], bf16, tag="warm")
        nc.vector.memset(warm[:, :], 0.0)
        nc.scalar.activation(warm[:, :], warm[:, :], AFT.Exp)

        iota = cpool.tile([128, 64], f32, tag="iota")
        nc.scalar.dma_start(iota[:, :], iota_in[:, :])

        # accumulators: col j = s*4 + e  (e = h//128 block)
        RS1 = apool.tile([128, 32], f32, tag="RS1")   # sum p1
        RSr = apool.tile([128, 32], f32, tag="RSr")   # sum r
        DM1 = apool.tile([128, 32], f32, tag="DM1")   # rowcount argmax==1
        DMA_ = apool.tile([128, 32], f32, tag="DMA")  # rowcount argmax in {1,2}
        for acc_t in (RS1, RSr, DM1, DMA_):
            nc.vector.memset(acc_t[:, :], 0.0)

        junkA = cpool.tile([128, 2048], bf16, tag="junkA")
        junkB = cpool.tile([128, 2048], bf16, tag="junkB")

        X = {}
        Ff = {}
        Rr = {}

        def eslice(base, e):
            return slice(base + e * 512, base + (e + 1) * 512)

        def esl(e):
            return slice(0, 2048) if e is None else eslice(0, e)

        def ld(s, c):
            mode = CFG["load"][s]
            if mode == "cast":
                return "pool", bf16
            if mode == "sp":
                return "sp", f32
            if mode == "mixed1":
                return ("pool", bf16) if c == 1 else ("sp", f32)
            return ("pool", bf16) if c == 2 else ("sp", f32)

        def load_plane(s, c, half=None):
            q, dt = ld(s, c)
            key = (s, c)
            if key not in X:
                pool_ = xcpool if dt == bf16 else xfpool
                X[key] = pool_.tile([128, NB, 512], dt,
                                    tag=f"X{'c' if dt == bf16 else 'f'}{c}",
                                    name=f"X_{s}_{c}")
            if half is None:
                src = x[s, c].rearrange("(e p) w -> p e w", p=128)
                QENG[q].dma_start(X[key], src)
            else:
                e0 = half * 2
                src = x[s, c, e0 * 128:(e0 + 2) * 128, :].rearrange(
                    "(e p) w -> p e w", p=128)
                QENG[q].dma_start(X[key][:, e0:e0 + 2], src)

        def fview(s, li):
            """AP of f_l (exp of t_l) as (128, 2048)."""
            if is_cast(s):
                return X[(s, li)].rearrange("p e w -> p (e w)")
            return Ff[s][:, (li - 1) * 2048:li * 2048]

        def csl(ch):
            if ch is None:
                return slice(0, 2048)
            e0, ne = ch
            return slice(e0 * 512, (e0 + ne) * 512)

        def stage_front(s, chunks=(None,)):
            """t halves built (in place for cast), then f = exp(t) in place."""
            cast = is_cast(s)
            if not cast and s not in Ff:
                Ff[s] = fpool.tile([128, 4096], bf16, tag="F", name=f"F_{s}")
            se = QENG[CFG["subs"][s]]
            x0 = X[(s, 0)].rearrange("p e w -> p (e w)")
            x1 = X[(s, 1)].rearrange("p e w -> p (e w)")
            x2 = X[(s, 2)].rearrange("p e w -> p (e w)")
            t1 = fview(s, 1)
            t2 = fview(s, 2)
            for ch in chunks:
                sl = csl(ch)
                se.tensor_tensor(t1[:, sl], x1[:, sl], x0[:, sl],
                                 Alu.subtract)
                se.tensor_tensor(t2[:, sl], x2[:, sl], x0[:, sl],
                                 Alu.subtract)
                if cast or ch is not None:
                    nc.scalar.activation(t1[:, sl], t1[:, sl], AFT.Exp)
                    nc.scalar.activation(t2[:, sl], t2[:, sl], AFT.Exp)
                else:
                    F = Ff[s]
                    nc.scalar.activation(F[:, :], F[:, :], AFT.Exp)

        def stage_mid(s, chunks=(None,)):
            """sadd = f1+f2; denominator r (+ sum r accumulated)."""
            if s not in Rr:
                Rr[s] = rpool.tile([128, 2048], bf16, tag="r", name=f"r_{s}")
                Rr[(s, "sadd")] = mpool.tile([128, 2048], bf16, tag="sadd",
                                             name=f"sa_{s}")
                Rr[(s, "aux")] = mpool.tile([128, 2048], bf16, tag="aux",
                                            name=f"aux_{s}")
            r, sadd, aux = Rr[s], Rr[(s, "sadd")], Rr[(s, "aux")]
            f1 = fview(s, 1)
            f2 = fview(s, 2)
            saddf = QENG[CFG["sadd"][s]]
            dmode = CFG["denom"][s]
            if dmode == "split" and chunks == (None,):
                chunks = [(0, 2), (2, 2)]
            for ch in chunks:
                sl = csl(ch)
                col = s * 4 + (0 if ch is None else ch[0])
                saddf.tensor_tensor(sadd[:, sl], f1[:, sl], f2[:, sl],
                                    Alu.add)
                dve_denom = (dmode == "dve"
                             or (dmode == "split" and ch is not None
                                 and ch[0] >= 2))
                if dve_denom:
                    nc.vector.tensor_scalar_add(aux[:, sl], sadd[:, sl], 1.0)
                    with nc.allow_low_precision(reason="bf16 softmax denom"):
                        nc.vector.reciprocal(r[:, sl], aux[:, sl])
                    nc.vector.tensor_scalar(
                        junkA[:, sl], r[:, sl], 1.0, 0.0, Alu.mult, Alu.add,
                        accum_out=RSr[:, col:col + 1])
                else:
                    nc.scalar.activation(aux[:, sl], sadd[:, sl], AFT.Ln,
                                         bias=1.0)
                    nc.scalar.activation(r[:, sl], aux[:, sl], AFT.Exp,
                                         scale=-1.0,
                                         accum_out=RSr[:, col:col + 1])

        def stage_back(s, chunks=(None,)):
            """p1 sums; m = max(f1,f2) + A counts; d1 counts (f-space)."""
            r = Rr[s]
            f1 = fview(s, 1)
            f2 = fview(s, 2)
            if (s, "pscr") not in Rr:
                Rr[(s, "pscr")] = bpool.tile([128, 2048], bf16, tag="pscr",
                                             name=f"p_{s}")
                Rr[(s, "mm")] = rpool.tile([128, 2048], bf16, tag="mm",
                                           name=f"mm_{s}")
            pscr, mm = Rr[(s, "pscr")], Rr[(s, "mm")]
            p1f = QENG[CFG["p1"][s]]
            mmf = QENG[CFG["mEng"][s]]
            d1_mode = CFG["d1"][s]
            if d1_mode == "B" and (s, "m1") not in Rr:
                Rr[(s, "m1")] = bpool.tile([128, 2048], bf16, tag="m1",
                                           name=f"m1_{s}")
                Rr[(s, "g1")] = bpool.tile([128, 2048], bf16, tag="g1",
                                           name=f"g1_{s}")
            for ch in chunks:
                sl = csl(ch)
                col0 = s * 4 + (0 if ch is None else ch[0])
                # p1 = f1 * r, row-sums into RS1
                p1f.tensor_tensor(pscr[:, sl], f1[:, sl], r[:, sl], Alu.mult)
                nc.vector.tensor_scalar(
                    junkB[:, sl], pscr[:, sl], 1.0, 0.0, Alu.mult, Alu.add,
                    accum_out=RS1[:, col0:col0 + 1])
                # m = max(f1, f2)
                mmf.tensor_tensor(mm[:, sl], f1[:, sl], f2[:, sl], Alu.max)
                if d1_mode == "B":
                    m1, g1 = Rr[(s, "m1")], Rr[(s, "g1")]
                    nc.vector.tensor_scalar_max(m1[:, sl], f2[:, sl], 1.0)
                    ge = QENG[CFG["d1_g_eng"].get(s, "pool")]
                    ge.tensor_tensor(g1[:, sl], f1[:, sl], m1[:, sl],
                                     Alu.subtract)
                es = (range(NB) if ch is None
                      else range(ch[0], ch[0] + ch[1]))
                for ee in es:
                    col = s * 4 + ee
                    # A = rowcount[max(f1,f2) > 1]
                    nc.vector.tensor_scalar(
                        junkA[:, eslice(0, ee)], mm[:, eslice(0, ee)],
                        1.0, 0.0, Alu.is_gt, Alu.add,
                        accum_out=DMA_[:, col:col + 1])
                    # d1 = rowcount[max(f2,1) < f1]
                    if d1_mode == "B":
                        nc.vector.tensor_scalar(
                            junkB[:, eslice(0, ee)],
                            Rr[(s, "g1")][:, eslice(0, ee)],
                            0.0, 0.0, Alu.is_gt, Alu.add,
                            accum_out=DM1[:, col:col + 1])
                    else:
                        nc.vector.scalar_tensor_tensor(
                            junkB[:, eslice(0, ee)], f2[:, eslice(0, ee)],
                            1.0, f1[:, eslice(0, ee)], Alu.max, Alu.is_lt,
                            accum_out=DM1[:, col:col + 1])

        # ---- software-pipelined emission ----
        WOVEN = set(CFG["weave"])

        def emit_loads(i):
            if i >= SPC:
                return
            if (i == 0 and CFG["chunk_head"]) or i in CFG.get("half_loads",
                                                              ()):
                for half in (0, 1):
                    for c in range(C):
                        load_plane(i, c, half=half)
                return
            if i in WOVEN:
                return  # interleaved below
            for c in range(C):
                load_plane(i, c)

        def emit_woven_loads(i):
            for s in sorted(WOVEN):
                st = CFG["il_start"][s]
                if st <= i <= st + 1:
                    h = i - st
                    for c in range(C):
                        load_plane(s, c, half=h)

        def chunked(j, stage):
            if j == 0 and CFG["chunk_head"]:
                return [(0, 1), (1, 1), (2, 2)]
            if j == 1 and CFG["chunk_s1"] and stage == "f":
                return [(0, 2), (2, 2)]
            if CFG.get("chunk_all"):
                return [(0, 2), (2, 2)]
            return (None,)

        def do_front(i):
            j = i - CFG["lag_front"]
            if 0 <= j < SPC and j not in WOVEN:
                stage_front(j, chunked(j, "f"))

        def do_mid(i):
            j = i - CFG["lag_mid"]
            if 0 <= j < SPC and j not in WOVEN:
                stage_mid(j, chunked(j, "m"))

        def do_back(i):
            j = i - CFG["lag_back"]
            if 0 <= j < SPC and j not in WOVEN:
                stage_back(j, chunked(j, "b"))

        # woven samples run chunk-granular front/mid/back
        def wch(s):
            return CFG.get("weave_ch", {}).get(s, [(0, 2), (2, 2)])

        def do_weave(i):
            for s in sorted(WOVEN):
                wl = CFG["weave_lag"][s]
                ch = wch(s)
                h = i - wl
                if 0 <= h < len(ch):
                    stage_front(s, (ch[h],))
                h = i - wl - 1
                if 0 <= h < len(ch):
                    stage_mid(s, (ch[h],))
                h = i - wl - 2
                if 0 <= h < len(ch):
                    stage_back(s, (ch[h],))

        n_iter = max([SPC + CFG["lag_back"] + 1]
                     + [CFG["weave_lag"][s] + len(wch(s)) + 3
                        for s in WOVEN])
        OMAP = {"b": do_back, "m": do_mid, "f": do_front}
        for i in range(n_iter):
            for ch in CFG["order"]:
                if ch == "l":
                    emit_loads(i)
                    emit_woven_loads(i)
                else:
                    OMAP[ch](i)
            do_weave(i)

        # ---- tail ----
        # column sums of the p1/r accumulators via gpsimd partition reduce
        O = cpool.tile([1, 40], f32, tag="O")
        S1 = cpool.tile([1, 64], f32, tag="S1")
        nc.gpsimd.tensor_reduce(S1[:, 0:32], RS1[:, :], C_AX, op=Alu.add)
        nc.gpsimd.tensor_reduce(S1[:, 32:64], RSr[:, :], C_AX, op=Alu.add)

        # DM2 = A - DM1
        teng = QENG[CFG.get("tail_tt", "dve")]
        DM2 = apool.tile([128, 32], f32, tag="DM2")
        teng.tensor_tensor(DM2[:, :], DMA_[:, :], DM1[:, :],
                           Alu.subtract)

        heights = []
        for li, DM in enumerate((DM1, DM2)):
            # pen = 1e6 where row absent; iota2[p, col] = (col%4)*128 + p
            pen = cpool.tile([128, 32], f32, tag=f"pen{li}")
            nc.vector.tensor_scalar(pen[:, :], DM[:, :], 0.5, 1e6,
                                    Alu.is_lt, Alu.mult)
            # ymin via max of negated iota: cols 32:64 of iota hold -h
            cminN = cpool.tile([128, 32], f32, tag=f"cminN{li}")
            teng.tensor_tensor(cminN[:, :], iota[:, 32:64], pen[:, :],
                               Alu.subtract)
            cmax = cpool.tile([128, 32], f32, tag=f"cmax{li}")
            teng.tensor_tensor(cmax[:, :], iota[:, 0:32], pen[:, :],
                               Alu.subtract)
            YminN = cpool.tile([1, 32], f32, tag=f"YminN{li}")
            Ymax = cpool.tile([1, 32], f32, tag=f"Ymax{li}")
            nc.gpsimd.tensor_reduce(YminN[:, :], cminN[:, :], C_AX,
                                    op=Alu.max)
            nc.gpsimd.tensor_reduce(Ymax[:, :], cmax[:, :], C_AX, op=Alu.max)
            yminN8 = cpool.tile([1, 8], f32, tag=f"yminN{li}")
            ymax8 = cpool.tile([1, 8], f32, tag=f"ymax{li}")
            nc.vector.tensor_reduce(
                yminN8[:, :],
                YminN[0:1, :].rearrange("p (s e) -> p s e", e=4),
                X_AX, op=Alu.max)
            nc.vector.tensor_reduce(
                ymax8[:, :],
                Ymax[0:1, :].rearrange("p (s e) -> p s e", e=4),
                X_AX, op=Alu.max)
            hL = cpool.tile([1, 8], f32, tag=f"h{li}")
            nc.vector.tensor_tensor(hL[:, :], ymax8[:, :], yminN8[:, :],
                                    Alu.add)
            nc.vector.tensor_scalar_max(hL[:, :], hL[:, :], 0.0)
            heights.append(hL)

        h_cup, h_disc = heights
        den = cpool.tile([1, 8], f32, tag="den")
        nc.vector.tensor_scalar_add(den[:, :], h_disc[:, :], 1e-6)
        rec = cpool.tile([1, 8], f32, tag="rec")
        nc.vector.reciprocal(rec[:, :], den[:, :])
        nc.vector.tensor_tensor(O[:, 0:8], h_cup[:, :], rec[:, :], Alu.mult)

        s1tot = cpool.tile([1, 8], f32, tag="s1tot")
        srtot = cpool.tile([1, 8], f32, tag="srtot")
        p2tot = cpool.tile([1, 8], f32, tag="p2tot")
        p2a = cpool.tile([1, 8], f32, tag="p2a")
        sc = 1.0 / HW

        nc.vector.tensor_reduce(
            s1tot[:, :],
            S1[0:1, 0:32].rearrange("p (s e) -> p s e", e=4),
            X_AX, op=Alu.add)
        nc.vector.tensor_reduce(
            srtot[:, :],
            S1[0:1, 32:64].rearrange("p (s e) -> p s e", e=4),
            X_AX, op=Alu.add)
        nc.vector.tensor_scalar(p2a[:, :], srtot[:, :], -1.0, HW,
                                Alu.mult, Alu.add)
        nc.vector.tensor_tensor(p2tot[:, :], p2a[:, :], s1tot[:, :],
                                Alu.subtract)
        nc.vector.tensor_scalar_mul(O[:, 8:16], p2tot[:, :], sc)
        nc.vector.tensor_scalar_mul(O[:, 16:24], s1tot[:, :], sc)
        nc.vector.tensor_scalar_mul(O[:, 24:32], p2tot[:, :], sc)
        nc.vector.tensor_scalar_mul(O[:, 32:40], s1tot[:, :], sc)

        nc.sync.dma_start(out[:, :], O[:, :])

    nc.finalize()
    return nc


def _get_nc():
    if "nc" not in _CACHE:
        _CACHE["nc"] = _build()
    return _CACHE["nc"]


def _host_inputs():
    # iota[p, s*4+e] = e*128 + p; cols 32:64 hold the negation
    iota = (np.arange(128, dtype=np.float32)[:, None]
            + 128.0 * np.tile(np.arange(4, dtype=np.float32), 8)[None, :])
    return (np.concatenate([iota, -iota], axis=1),)


def _run(seg_mask, trace=False):
    from concourse.bass_utils import run_bass_kernel_spmd

    x = np.ascontiguousarray(np.asarray(seg_mask, dtype=np.float32))
    assert x.shape == (B, C, H, W)
    (iota,) = _host_inputs()
    in_maps = [
        {"x": x[SPC * c:SPC * (c + 1)], "iota": iota}
        for c in range(NCORES)
    ]
    nc = _get_nc()
    res = run_bass_kernel_spmd(nc, in_maps, core_ids=list(range(NCORES)),
                               trace=trace)
    outs = []
    for c in range(NCORES):
        o = np.asarray(res.results[c]["out"]).reshape(5, SPC).T
        outs.append(o)
    full = np.concatenate(outs, axis=0).astype(np.float32)
    return full, res


def kernel(segmentation_mask):
    full, _ = _run(segmentation_mask, trace=False)
    return full


# revision 34
# speedup vs baseline: 1.0111x; 1.0111x over previous
"""Trainium2 Bass kernel for CDRExtractor (segment_reduce).

Input : segmentation_mask (64, 3, 512, 512) fp32
Output: (64, 5) fp32 = [cdr, disc_mean, cup_mean, disc_mean, cup_mean]

Sharding: pure data parallel, 8 samples per core across 8 cores.

v3 design, 68.4us (v2 was 76.8us).  What changed and why:
  - SWDGE (gpsimd) casting DMA loads fp32->bf16 at HALF the queue cost
    (the cost model prices a DMA by OUTPUT bytes per partition, and the
    CoreSim model charges each DMA to its issuing engine queue).  The
    24-plane shard splits: ~14 planes fp32 on the SP queue, ~10 planes
    bf16-cast on the Pool queue (samples 0,3 fully cast; 2,4,6 "mixed"
    with x2 cast).  DMA queue time drops 75.8us -> ~62us total.
  - Everything after exp works in f-space (exp is monotonic):
    d1 = rowcount[f1 > max(f2,1)], A = rowcount[max(f1,f2) > 1], and
    d2 = A - d1 exactly (A counts argmax in {1,2}).  A's count is a 4x
    tensor_scalar accumulate off a 2x TT max, which replaces d2's 1x
    fused STT.  f-space also lets exp run IN PLACE over the subtract
    output (cast samples: in place over the X tiles), killing the
    separate T tiles and ~30% of SBUF traffic/pressure.
  - ACT does the exps (one 4096-wide instr per unchunked sample) plus
    ln/exp denominators for ~5 samples; the other denominators run on
    DVE (ts_add 4x + reciprocal 1x + sum_r accumulate 4x).  Consts DMA
    and the act-table warm hide in ACT's idle ramp.
  - Real-ISA limits found on the axon path (the cost model is laxer):
    Pool/gpsimd cannot run TensorScalarPtr at all (no tensor_scalar /
    scalar_tensor_tensor / accum_out) and Pool TT supports only
    add/subtract/mult (no max); TT divide and DMA cast+accum are
    rejected everywhere.  So every accumulation (p1, sum_r, A, d1) and
    every max lives on DVE; Pool gets subs/sadd/p1-mult/g-subs.
  - Tail without PE: gpsimd tensor_reduce(C axis) does the partition
    sums/maxes directly (add/avg/max only - ymin is computed as a max
    of negated iota columns), removing both PE transposes, the
    identity/ones consts, and PSUM.  heights: rowcount>0.5 penalty +
    iota min/max as before; cdr = h_cup/(h_disc+1e-6); means via
    sum p1 and the identity sum p2 = HW - sum r - sum p1.
  - Schedule: software pipeline with lags front/mid/back = 2/3/4,
    per-iteration emission order back,mid,front,loads; s0 chunked at
    [e0,e1,(e2e3)] for the ramp; s1's front at halves; samples 5,6,7
    woven at half granularity near the drain with their loads
    interleaved from iterations 3-6.  Engine busy (CoreSim): Pool 60.6
    / DVE 57.4 / ACT 57.3 / SP 47.9; makespan 68.4us.

Numerics: bf16 internals, fp32 accumulators.  Rel err vs fp32
reference 2.05e-04 (gate 2e-2), HW-verified via test.py on the axon
run path: HW exec 68395 ns.
"""

import numpy as np
from contextlib import ExitStack

B, C, H, W = 64, 3, 512, 512
NCORES = 8
SPC = B // NCORES      # samples per core = 8
NB = H // 128          # 128-row blocks = 4
HW = float(H * W)

_CACHE = {}

# ---- per-sample engine/config tables ----
CFG = dict(
    # load: 'cast' (Pool SWDGE bf16) | 'sp' (fp32 on SP) | 'mixed'
    # (x0,x1 fp32 on SP; x2 cast on Pool)
    load={0: "cast", 1: "sp", 2: "mixed", 3: "cast", 4: "mixed", 5: "sp",
          6: "mixed", 7: "sp"},
    denom={0: "dve", 1: "act", 2: "act", 3: "dve", 4: "act", 5: "split",
           6: "act", 7: "act"},
    subs={0: "dve", 1: "pool", 2: "pool", 3: "dve", 4: "pool",
          5: "pool", 6: "pool", 7: "pool"},
    sadd={0: "dve", 1: "pool", 2: "pool", 3: "dve", 4: "pool",
          5: "pool", 6: "pool", 7: "pool"},
    p1={0: "pool", 1: "dve", 2: "pool", 3: "pool", 4: "dve", 5: "dve",
        6: "pool", 7: "dve"},
    mEng={0: "dve", 1: "dve", 2: "dve", 3: "dve", 4: "dve", 5: "dve",
          6: "dve", 7: "dve"},
    # d1 mode: 'fused' (DVE STT 1x) | 'B' (DVE ts_max + g-sub TT + is_gt)
    d1={0: "fused", 1: "B", 2: "fused", 3: "fused", 4: "B", 5: "B",
        6: "B", 7: "B"},
    d1_g_eng={1: "pool", 4: "pool", 5: "pool", 6: "pool", 7: "pool"},
    lag_front=2, lag_mid=3, lag_back=4,
    chunk_head=True,     # s0 loads/front at half-plane granularity
    chunk_s1=True,       # s1 subs/exps at half granularity (ramp)
    weave=(5, 6, 7),     # samples staged at half granularity near the end
    weave_lag={5: 5, 6: 6, 7: 7},
    il_start={5: 3, 6: 4, 7: 5},   # iteration when woven loads interleave
    order="bmfl",        # emission order within an iteration
    tail_tt="pool",      # engine for tail TT ops
    p1_last=(7,),        # samples whose p1 ops emit after the d-counts
)


def _build():
    import concourse.bass as bass
    import concourse.bacc as bacc
    import concourse.mybir as mybir
    from concourse.tile import TileContext

    if not _CACHE.get("act_patch"):
        _orig_tables = bacc.get_activation_tables

        def _only_ln_exp(arch):
            t = _orig_tables(arch)
            keep = "natural_log_exp_and_others"
            return {k: (v if k == keep else set()) for k, v in t.items()}

        bacc.get_activation_tables = _only_ln_exp
        _CACHE["act_patch"] = True

    f32 = mybir.dt.float32
    bf16 = mybir.dt.bfloat16
    Alu = mybir.AluOpType
    AFT = mybir.ActivationFunctionType
    X_AX = mybir.AxisListType.X
    C_AX = mybir.AxisListType.C

    nc = bacc.Bacc()
    x = nc.dram_tensor("x", (SPC, C, H, W), f32, kind="ExternalInput")
    iota_in = nc.dram_tensor("iota", (128, 64), f32, kind="ExternalInput")
    out = nc.dram_tensor("out", (5, SPC), f32, kind="ExternalOutput")

    s_last = SPC - 1

    def is_cast(s):
        return CFG["load"][s] == "cast"

    with TileContext(nc) as tc, ExitStack() as ctx:
        QENG = dict(sp=nc.sync, act=nc.scalar, pool=nc.gpsimd,
                    dve=nc.vector)
        cpool = ctx.enter_context(tc.tile_pool(name="consts", bufs=1))
        apool = ctx.enter_context(tc.tile_pool(name="accs", bufs=1))
        xcpool = ctx.enter_context(tc.tile_pool(name="xc", bufs=3))
        xfpool = ctx.enter_context(tc.tile_pool(name="xf", bufs=2))
        fpool = ctx.enter_context(tc.tile_pool(name="fmain", bufs=3))
        mpool = ctx.enter_context(tc.tile_pool(name="mid", bufs=3))
        bpool = ctx.enter_context(tc.tile_pool(name="bck", bufs=2))
        rpool = ctx.enter_context(tc.tile_pool(name="rr", bufs=3))

        # act-table warm + consts ride the ACT queue's idle ramp
        warm = cpool.tile([1, 16], bf16, tag="warm")
        nc.vector.memset(warm[:, :], 0.0)
        nc.scalar.activation(warm[:, :], warm[:, :], AFT.Exp)

        iota = cpool.tile([128, 64], f32, tag="iota")
        nc.scalar.dma_start(iota[:, :], iota_in[:, :])

        # accumulators: col j = s*4 + e  (e = h//128 block)
        RS1 = apool.tile([128, 32], f32, tag="RS1")   # sum p1
        RSr = apool.tile([128, 32], f32, tag="RSr")   # sum r
        DM1 = apool.tile([128, 32], f32, tag="DM1")   # rowcount argmax==1
        DMA_ = apool.tile([128, 32], f32, tag="DMA")  # rowcount argmax in {1,2}
        for acc_t in (RS1, RSr, DM1, DMA_):
            nc.vector.memset(acc_t[:, :], 0.0)

        junkA = cpool.tile([128, 2048], bf16, tag="junkA")
        junkB = cpool.tile([128, 2048], bf16, tag="junkB")

        X = {}
        Ff = {}
        Rr = {}

        def eslice(base, e):
            return slice(base + e * 512, base + (e + 1) * 512)

        def esl(e):
            return slice(0, 2048) if e is None else eslice(0, e)

        def ld(s, c):
            mode = CFG["load"][s]
            if mode == "cast":
                return "pool", bf16
            if mode == "sp":
                return "sp", f32
            if mode == "mixed1":
                return ("pool", bf16) if c == 1 else ("sp", f32)
            return ("pool", bf16) if c == 2 else ("sp", f32)

        def load_plane(s, c, half=None):
            q, dt = ld(s, c)
            key = (s, c)
            if key not in X:
                pool_ = xcpool if dt == bf16 else xfpool
                X[key] = pool_.tile([128, NB, 512], dt,
                                    tag=f"X{'c' if dt == bf16 else 'f'}{c}",
                                    name=f"X_{s}_{c}")
            if half is None:
                src = x[s, c].rearrange("(e p) w -> p e w", p=128)
                QENG[q].dma_start(X[key], src)
            else:
                e0 = half * 2
                src = x[s, c, e0 * 128:(e0 + 2) * 128, :].rearrange(
                    "(e p) w -> p e w", p=128)
                QENG[q].dma_start(X[key][:, e0:e0 + 2], src)

        def fview(s, li):
            """AP of f_l (exp of t_l) as (128, 2048)."""
            if is_cast(s):
                return X[(s, li)].rearrange("p e w -> p (e w)")
            return Ff[s][:, (li - 1) * 2048:li * 2048]

        def csl(ch):
            if ch is None:
                return slice(0, 2048)
            e0, ne = ch
            return slice(e0 * 512, (e0 + ne) * 512)

        def stage_front(s, chunks=(None,)):
            """t halves built (in place for cast), then f = exp(t) in place."""
            cast = is_cast(s)
            if not cast and s not in Ff:
                Ff[s] = fpool.tile([128, 4096], bf16, tag="F", name=f"F_{s}")
            se = QENG[CFG["subs"][s]]
            x0 = X[(s, 0)].rearrange("p e w -> p (e w)")
            x1 = X[(s, 1)].rearrange("p e w -> p (e w)")
            x2 = X[(s, 2)].rearrange("p e w -> p (e w)")
            t1 = fview(s, 1)
            t2 = fview(s, 2)
            for ch in chunks:
                sl = csl(ch)
                se.tensor_tensor(t1[:, sl], x1[:, sl], x0[:, sl],
                                 Alu.subtract)
                se.tensor_tensor(t2[:, sl], x2[:, sl], x0[:, sl],
                                 Alu.subtract)
                if cast or ch is not None:
                    nc.scalar.activation(t1[:, sl], t1[:, sl], AFT.Exp)
                    nc.scalar.activation(t2[:, sl], t2[:, sl], AFT.Exp)
                else:
                    F = Ff[s]
                    nc.scalar.activation(F[:, :], F[:, :], AFT.Exp)

        def stage_mid(s, chunks=(None,)):
            """sadd = f1+f2; denominator r (+ sum r accumulated)."""
            if s not in Rr:
                Rr[s] = rpool.tile([128, 2048], bf16, tag="r", name=f"r_{s}")
                Rr[(s, "sadd")] = mpool.tile([128, 2048], bf16, tag="sadd",
                                             name=f"sa_{s}")
                Rr[(s, "aux")] = mpool.tile([128, 2048], bf16, tag="aux",
                                            name=f"aux_{s}")
            r, sadd, aux = Rr[s], Rr[(s, "sadd")], Rr[(s, "aux")]
            f1 = fview(s, 1)
            f2 = fview(s, 2)
            saddf = QENG[CFG["sadd"][s]]
            dmode = CFG["denom"][s]
            if dmode == "split" and chunks == (None,):
                chunks = [(0, 2), (2, 2)]
            for ch in chunks:
                sl = csl(ch)
                col = s * 4 + (0 if ch is None else ch[0])
                saddf.tensor_tensor(sadd[:, sl], f1[:, sl], f2[:, sl],
                                    Alu.add)
                dve_denom = (dmode == "dve"
                             or (dmode == "split" and ch is not None
                                 and ch[0] >= 2))
                if dve_denom:
                    nc.vector.tensor_scalar_add(aux[:, sl], sadd[:, sl], 1.0)
                    with nc.allow_low_precision(reason="bf16 softmax denom"):
                        nc.vector.reciprocal(r[:, sl], aux[:, sl])
                    nc.vector.tensor_scalar(
                        junkA[:, sl], r[:, sl], 1.0, 0.0, Alu.mult, Alu.add,
                        accum_out=RSr[:, col:col + 1])
                else:
                    nc.scalar.activation(aux[:, sl], sadd[:, sl], AFT.Ln,
                                         bias=1.0)
                    nc.scalar.activation(r[:, sl], aux[:, sl], AFT.Exp,
                                         scale=-1.0,
                                         accum_out=RSr[:, col:col + 1])

        def stage_back(s, chunks=(None,)):
            """p1 sums; m = max(f1,f2) + A counts; d1 counts (f-space)."""
            r = Rr[s]
            f1 = fview(s, 1)
            f2 = fview(s, 2)
            if (s, "pscr") not in Rr:
                Rr[(s, "pscr")] = bpool.tile([128, 2048], bf16, tag="pscr",
                                             name=f"p_{s}")
                Rr[(s, "mm")] = rpool.tile([128, 2048], bf16, tag="mm",
                                           name=f"mm_{s}")
            pscr, mm = Rr[(s, "pscr")], Rr[(s, "mm")]
            p1f = QENG[CFG["p1"][s]]
            mmf = QENG[CFG["mEng"][s]]
            d1_mode = CFG["d1"][s]
            if d1_mode == "B" and (s, "m1") not in Rr:
                Rr[(s, "m1")] = bpool.tile([128, 2048], bf16, tag="m1",
                                           name=f"m1_{s}")
                Rr[(s, "g1")] = bpool.tile([128, 2048], bf16, tag="g1",
                                           name=f"g1_{s}")
            p1_last = s in CFG.get("p1_last", ())
            for ch in chunks:
                sl = csl(ch)
                col0 = s * 4 + (0 if ch is None else ch[0])

                def emit_p1():
                    # p1 = f1 * r, row-sums into RS1
                    p1f.tensor_tensor(pscr[:, sl], f1[:, sl], r[:, sl],
                                      Alu.mult)
                    nc.vector.tensor_scalar(
                        junkB[:, sl], pscr[:, sl], 1.0, 0.0, Alu.mult,
                        Alu.add, accum_out=RS1[:, col0:col0 + 1])

                if not p1_last:
                    emit_p1()
                # m = max(f1, f2)
                mmf.tensor_tensor(mm[:, sl], f1[:, sl], f2[:, sl], Alu.max)
                if d1_mode == "B":
                    m1, g1 = Rr[(s, "m1")], Rr[(s, "g1")]
                    nc.vector.tensor_scalar_max(m1[:, sl], f2[:, sl], 1.0)
                    ge = QENG[CFG["d1_g_eng"].get(s, "pool")]
                    ge.tensor_tensor(g1[:, sl], f1[:, sl], m1[:, sl],
                                     Alu.subtract)
                es = (range(NB) if ch is None
                      else range(ch[0], ch[0] + ch[1]))
                for ee in es:
                    col = s * 4 + ee
                    # A = rowcount[max(f1,f2) > 1]
                    nc.vector.tensor_scalar(
                        junkA[:, eslice(0, ee)], mm[:, eslice(0, ee)],
                        1.0, 0.0, Alu.is_gt, Alu.add,
                        accum_out=DMA_[:, col:col + 1])
                    # d1 = rowcount[max(f2,1) < f1]
                    if d1_mode == "B":
                        nc.vector.tensor_scalar(
                            junkB[:, eslice(0, ee)],
                            Rr[(s, "g1")][:, eslice(0, ee)],
                            0.0, 0.0, Alu.is_gt, Alu.add,
                            accum_out=DM1[:, col:col + 1])
                    else:
                        nc.vector.scalar_tensor_tensor(
                            junkB[:, eslice(0, ee)], f2[:, eslice(0, ee)],
                            1.0, f1[:, eslice(0, ee)], Alu.max, Alu.is_lt,
                            accum_out=DM1[:, col:col + 1])
                if p1_last:
                    emit_p1()

        # ---- software-pipelined emission ----
        WOVEN = set(CFG["weave"])

        def emit_loads(i):
            if i >= SPC:
                return
            if (i == 0 and CFG["chunk_head"]) or i in CFG.get("half_loads",
                                                              ()):
                for half in (0, 1):
                    for c in range(C):
                        load_plane(i, c, half=half)
                return
            if i in WOVEN:
                return  # interleaved below
            for c in range(C):
                load_plane(i, c)

        def emit_woven_loads(i):
            for s in sorted(WOVEN):
                st = CFG["il_start"][s]
                if st <= i <= st + 1:
                    h = i - st
                    for c in range(C):
                        load_plane(s, c, half=h)

        def chunked(j, stage):
            if j == 0 and CFG["chunk_head"]:
                return [(0, 1), (1, 1), (2, 2)]
            if j == 1 and CFG["chunk_s1"] and stage == "f":
                return [(0, 2), (2, 2)]
            if CFG.get("chunk_all"):
                return [(0, 2), (2, 2)]
            return (None,)

        def do_front(i):
            j = i - CFG["lag_front"]
            if 0 <= j < SPC and j not in WOVEN:
                stage_front(j, chunked(j, "f"))

        def do_mid(i):
            j = i - CFG["lag_mid"]
            if 0 <= j < SPC and j not in WOVEN:
                stage_mid(j, chunked(j, "m"))

        def do_back(i):
            j = i - CFG["lag_back"]
            if 0 <= j < SPC and j not in WOVEN:
                stage_back(j, chunked(j, "b"))

        # woven samples run chunk-granular front/mid/back
        def wch(s):
            return CFG.get("weave_ch", {}).get(s, [(0, 2), (2, 2)])

        def do_weave(i):
            for s in sorted(WOVEN):
                wl = CFG["weave_lag"][s]
                ch = wch(s)
                h = i - wl
                if 0 <= h < len(ch):
                    stage_front(s, (ch[h],))
                h = i - wl - 1
                if 0 <= h < len(ch):
                    stage_mid(s, (ch[h],))
                h = i - wl - 2
                if 0 <= h < len(ch):
                    stage_back(s, (ch[h],))

        n_iter = max([SPC + CFG["lag_back"] + 1]
                     + [CFG["weave_lag"][s] + len(wch(s)) + 3
                        for s in WOVEN])
        OMAP = {"b": do_back, "m": do_mid, "f": do_front}
        for i in range(n_iter):
            for ch in CFG["order"]:
                if ch == "l":
                    emit_loads(i)
                    emit_woven_loads(i)
                else:
                    OMAP[ch](i)
            do_weave(i)

        # ---- tail ----
        # column sums of the p1/r accumulators via gpsimd partition reduce
        # (r sums first so the downstream [p2tot|s1tot] pair lands adjacent)
        O = cpool.tile([1, 40], f32, tag="O")
        S1 = cpool.tile([1, 64], f32, tag="S1")
        nc.gpsimd.tensor_reduce(S1[:, 0:32], RSr[:, :], C_AX, op=Alu.add)
        nc.gpsimd.tensor_reduce(S1[:, 32:64], RS1[:, :], C_AX, op=Alu.add)

        # DM2 = A - DM1
        teng = QENG[CFG.get("tail_tt", "dve")]
        DM2 = apool.tile([128, 32], f32, tag="DM2")
        teng.tensor_tensor(DM2[:, :], DMA_[:, :], DM1[:, :],
                           Alu.subtract)

        heights = []
        for li, DM in enumerate((DM1, DM2)):
            # pen = 1e6 where row absent; iota2[p, col] = (col%4)*128 + p
            pen = cpool.tile([128, 32], f32, tag=f"pen{li}")
            nc.vector.tensor_scalar(pen[:, :], DM[:, :], 0.5, 1e6,
                                    Alu.is_lt, Alu.mult)
            # ymin via max of negated iota: cols 32:64 of iota hold -h
            cminN = cpool.tile([128, 32], f32, tag=f"cminN{li}")
            teng.tensor_tensor(cminN[:, :], iota[:, 32:64], pen[:, :],
                               Alu.subtract)
            cmax = cpool.tile([128, 32], f32, tag=f"cmax{li}")
            teng.tensor_tensor(cmax[:, :], iota[:, 0:32], pen[:, :],
                               Alu.subtract)
            YminN = cpool.tile([1, 32], f32, tag=f"YminN{li}")
            Ymax = cpool.tile([1, 32], f32, tag=f"Ymax{li}")
            nc.gpsimd.tensor_reduce(YminN[:, :], cminN[:, :], C_AX,
                                    op=Alu.max)
            nc.gpsimd.tensor_reduce(Ymax[:, :], cmax[:, :], C_AX, op=Alu.max)
            yminN8 = cpool.tile([1, 8], f32, tag=f"yminN{li}")
            ymax8 = cpool.tile([1, 8], f32, tag=f"ymax{li}")
            nc.vector.tensor_reduce(
                yminN8[:, :],
                YminN[0:1, :].rearrange("p (s e) -> p s e", e=4),
                X_AX, op=Alu.max)
            nc.vector.tensor_reduce(
                ymax8[:, :],
                Ymax[0:1, :].rearrange("p (s e) -> p s e", e=4),
                X_AX, op=Alu.max)
            hs = cpool.tile([1, 8], f32, tag=f"hs{li}")
            nc.vector.tensor_tensor(hs[:, :], ymax8[:, :], yminN8[:, :],
                                    Alu.add)
            hL = cpool.tile([1, 8], f32, tag=f"h{li}")
            # disc height clamps at 1e-6: this IS the +1e-6 of the cdr
            # denominator (for h>0, adding 1e-6 is below fp32 ulp anyway)
            lo = 1e-6 if li == 1 else 0.0
            nc.vector.tensor_scalar(hL[:, :], hs[:, :], 0.0, lo,
                                    Alu.add, Alu.max)
            heights.append(hL)

        h_cup, h_disc = heights
        rec = cpool.tile([1, 8], f32, tag="rec")
        nc.vector.reciprocal(rec[:, :], h_disc[:, :])
        nc.vector.tensor_tensor(O[:, 0:8], h_cup[:, :], rec[:, :], Alu.mult)

        # pts = [srtot | s1tot] via one grouped reduce; p2tot overwrites
        # srtot's slot so [p2tot | s1tot] feeds both O pair-writes directly
        pts = cpool.tile([1, 16], f32, tag="pts")
        q8 = cpool.tile([1, 8], f32, tag="q8")
        sc = 1.0 / HW
        nc.vector.tensor_reduce(
            pts[:, :],
            S1[0:1, :].rearrange("p (s e) -> p s e", e=4),
            X_AX, op=Alu.add)
        nc.vector.tensor_tensor(q8[:, :], pts[:, 0:8], pts[:, 8:16],
                                Alu.add)
        nc.vector.tensor_scalar(pts[:, 0:8], q8[:, :], -1.0, HW,
                                Alu.mult, Alu.add)
        nc.vector.tensor_scalar_mul(O[:, 8:24], pts[:, :], sc)
        nc.vector.tensor_scalar_mul(O[:, 24:40], pts[:, :], sc)

        nc.sync.dma_start(out[:, :], O[:, :])

    nc.finalize()
    return nc


def _get_nc():
    if "nc" not in _CACHE:
        _CACHE["nc"] = _build()
    return _CACHE["nc"]


def _host_inputs():
    # iota[p, s*4+e] = e*128 + p; cols 32:64 hold the negation
    iota = (np.arange(128, dtype=np.float32)[:, None]
            + 128.0 * np.tile(np.arange(4, dtype=np.float32), 8)[None, :])
    return (np.concatenate([iota, -iota], axis=1),)


def _run(seg_mask, trace=False):
    from concourse.bass_utils import run_bass_kernel_spmd

    x = np.ascontiguousarray(np.asarray(seg_mask, dtype=np.float32))
    assert x.shape == (B, C, H, W)
    (iota,) = _host_inputs()
    in_maps = [
        {"x": x[SPC * c:SPC * (c + 1)], "iota": iota}
        for c in range(NCORES)
    ]
    nc = _get_nc()
    res = run_bass_kernel_spmd(nc, in_maps, core_ids=list(range(NCORES)),
                               trace=trace)
    outs = []
    for c in range(NCORES):
        o = np.asarray(res.results[c]["out"]).reshape(5, SPC).T
        outs.append(o)
    full = np.concatenate(outs, axis=0).astype(np.float32)
    return full, res


def kernel(segmentation_mask):
    full, _ = _run(segmentation_mask, trace=False)
    return full
